# revision 17
# baseline (speedup 1.0000x reference)
"""Trainium2 Bass kernel for nn_CustomGINConv (gnn_message_passing).

Reference computation (per path n, L=6 layers, C=128 channels):
    h[l]    = x[l] @ Wt[:C] + emb[idx[l]] @ Wt[C:] + bt
    prop[l] = h[l-1] + h[l+1]                (zero-padded along l)
    u[l]    = (1+eps) * x[l] + prop[l]
    out     = sum_l relu(u[l] @ W1 + b1) @ W2 + b2   -> [N, C]

Kernel strategy (shard N across 8 cores, feature-major on-chip layout):
  * Everything linear before the relu is folded host-side. With
    T = emb @ Wt[C:] + bt and s = 1+eps (the eps scales cancel):
      z1[l] = x[l] @ (s*W1) + (x[l-1]+x[l+1]) @ (Wt[:C] @ W1)
              + ohsum[l] @ (T @ W1)
    where ohsum[l] = onehot(idx[l-1]) + onehot(idx[l+1]).
    Then out = sum_l relu(z1[l] + b1) @ W2 + L*b2.
  * The propagate shift-add is absorbed into PSUM accumulation plus one
    stacked DVE add (xs[l] = x[l-1]+x[l+1]).
  * The embedding gather is a one-hot matmul: the host precomputes ohsum
    in fp8e4m3 with values {0, 1/16, 2/16} (exact) against a 16x-scaled
    (T @ W1) table so its small entries stay in fp8's normal range.
  * W2 distributes over the layer sum: relu outputs (bf16) are summed by
    a 5-add tree spread across DVE/Pool, so only ONE W2 matmul per tile
    runs instead of 6 — PE work drops 24->19.5 (DR) bank-passes/tile.
  * x path, relu outputs, W2 and the output are all bf16 (halves HBM
    payload on x and out; ~0.4% relative output error, budget 2e-2).
  * Input/output DMAs are split along L and issued from different engine
    queues (SP/ACT/Pool) so no single sequencer serializes the traffic.
  * KERNEL_DOUBLEROW=1 (default) uses fp8 DoubleRow perf mode for the
    one-hot matmul (halves its PE cycles; same HBM bytes).
"""

import os
import sys

import numpy as np

sys.path.insert(0, "/opt/trn_rl_repo")

import ml_dtypes  # noqa: E402

import concourse.bass as bass  # noqa: E402
import concourse.tile as tile  # noqa: E402
from concourse import bacc, mybir  # noqa: E402
from concourse import bass_utils  # noqa: E402
from concourse.bass import ts  # noqa: E402

L = 6
N_FULL = 65536
C = 128
EMB = 100
NCORES = 8
NC_N = N_FULL // NCORES  # 8192 rows per core
M = 512  # tile width (columns of the feature-major layout)

F32 = mybir.dt.float32
F32R = mybir.dt.float32r
BF16 = mybir.dt.bfloat16
F8 = mybir.dt.float8e4

RELU = mybir.ActivationFunctionType.Relu
IDENT = mybir.ActivationFunctionType.Identity

# fp8e4m3 bit patterns for {0, 1/16, 2/16}: the one-hot carries a 1/16
# factor (exact powers of two) and tw1 is pre-scaled by 16 so its small
# entries sit in fp8's normal range instead of the subnormals.
_FP8_LUT = np.array([0x00, 0x18, 0x20], dtype=np.uint8)
_OH_SCALE = np.float32(16.0)
DOUBLE_ROW = os.environ.get("KERNEL_DOUBLEROW", "1") == "1"
EMB_H = EMB // 2


def build_bass(nc_n: int = NC_N, num_devices: int = NCORES,
               repeat: int = 1) -> bass.Bass:
    """Build + compile the per-core Bass program (same program on all cores).

    repeat>1 re-runs the whole tile loop (for timing: on-device work scales
    by `repeat` while dispatch overhead stays fixed)."""
    nc = bacc.Bacc(
        "TRN2",
        target_bir_lowering=False,
        debug=False,
        enable_asserts=False,
        num_devices=num_devices,
    )
    LH = L // 2
    xt = nc.dram_tensor("xt", [C, L, nc_n], BF16, kind="ExternalInput").ap()
    if DOUBLE_ROW:
        # one-hot split along L into two tensors: the DoubleRow layout can't
        # be L-sliced in one AP (>3 dims after balancing), but two tensors
        # give two independent 1186ns DMAs instead of one 2372ns hold
        oh_lo = nc.dram_tensor(
            "oh_lo", [EMB_H, 2, LH, nc_n], F8, kind="ExternalInput"
        ).ap()
        oh_hi = nc.dram_tensor(
            "oh_hi", [EMB_H, 2, L - LH, nc_n], F8, kind="ExternalInput"
        ).ap()
        tw1 = nc.dram_tensor("tw1", [EMB_H, 2, C], F8, kind="ExternalInput").ap()
    else:
        oh_lo = nc.dram_tensor(
            "oh_lo", [EMB, LH, nc_n], F8, kind="ExternalInput"
        ).ap()
        oh_hi = nc.dram_tensor(
            "oh_hi", [EMB, L - LH, nc_n], F8, kind="ExternalInput"
        ).ap()
        tw1 = nc.dram_tensor("tw1", [EMB, C], F8, kind="ExternalInput").ap()
    w1d = nc.dram_tensor("w1d", [C, C], BF16, kind="ExternalInput").ap()
    w1x = nc.dram_tensor("w1x", [C, C], BF16, kind="ExternalInput").ap()
    w2 = nc.dram_tensor("w2", [C, C], BF16, kind="ExternalInput").ap()
    b1 = nc.dram_tensor("b1", [C, 1], F32, kind="ExternalInput").ap()
    b2s = nc.dram_tensor("b2s", [C, 1], F32, kind="ExternalInput").ap()
    out = nc.dram_tensor("out", [C, nc_n], BF16, kind="ExternalOutput").ap()

    nt = nc_n // M
    with tile.TileContext(nc) as tc:
        with (
            tc.tile_pool(name="consts", bufs=1) as consts,
            tc.tile_pool(name="xp", bufs=3) as xp,
            tc.tile_pool(name="ohp", bufs=3) as ohp,
            tc.tile_pool(name="rp", bufs=2) as rp,
            tc.tile_pool(name="outp", bufs=2) as outp,
            tc.tile_pool(name="pp", bufs=1, space="PSUM") as pp,
        ):
            # matmul-critical consts lead the SP queue; the late-needed ones
            # (b1/w2/b2) are emitted inside tile 0 after ACT's x loads
            w1d_sb = consts.tile([C, C], BF16, tag="w1d")
            nc.sync.dma_start(w1d_sb[:], w1d)
            tw1_sb = consts.tile(
                [EMB_H, 2, C] if DOUBLE_ROW else [EMB, C], F8, tag="tw1"
            )
            nc.sync.dma_start(tw1_sb[:], tw1)
            w1x_sb = consts.tile([C, C], BF16, tag="w1x")
            nc.sync.dma_start(w1x_sb[:], w1x)
            w2_sb = consts.tile([C, C], BF16, tag="w2")
            b1_sb = consts.tile([C, 1], F32, tag="b1")
            b2_sb = consts.tile([C, 1], F32, tag="b2")

            for i_rep in range(repeat * nt):
                i = i_rep % nt
                xt_t = xp.tile([C, L, M], BF16, tag="xt")
                oh_t = ohp.tile(
                    [EMB_H, 2, L, M] if DOUBLE_ROW else [EMB, L, M],
                    F8, tag="oh",
                )
                if i_rep == 0:
                    # split the very first loads per layer so l=0's matmuls
                    # start as soon as x[0], x[1], ohsum[0] land instead of
                    # waiting for the full tile (Pool leads with even layers:
                    # ACT's queue opens with its activation-table load)
                    for l in range(L):
                        eng = nc.gpsimd if l % 2 == 0 else nc.scalar
                        eng.dma_start(xt_t[:, l, :], xt[:, l, ts(i, M)])
                        src = (oh_lo, l) if l < LH else (oh_hi, l - LH)
                        if DOUBLE_ROW:
                            nc.sync.dma_start(
                                oh_t[:, :, l, :], src[0][:, :, src[1], ts(i, M)]
                            )
                        else:
                            nc.sync.dma_start(
                                oh_t[:, l, :], src[0][:, src[1], ts(i, M)]
                            )
                    nc.scalar.dma_start(b1_sb[:], b1)
                    nc.scalar.dma_start(w2_sb[:], w2)
                    nc.scalar.dma_start(b2_sb[:], b2s)
                else:
                    # spread DMA seq-hold across engine queues: SP carries the
                    # one-hot halves plus one x third; ACT and Pool take one
                    # x third each
                    nc.sync.dma_start(xt_t[:, 0:2, :], xt[:, 0:2, ts(i, M)])
                    nc.scalar.dma_start(xt_t[:, 2:4, :], xt[:, 2:4, ts(i, M)])
                    nc.gpsimd.dma_start(xt_t[:, 4:6, :], xt[:, 4:6, ts(i, M)])
                    if DOUBLE_ROW:
                        nc.sync.dma_start(
                            oh_t[:, :, :LH, :], oh_lo[:, :, :, ts(i, M)]
                        )
                        nc.sync.dma_start(
                            oh_t[:, :, LH:, :], oh_hi[:, :, :, ts(i, M)]
                        )
                    else:
                        nc.sync.dma_start(oh_t[:, :LH, :], oh_lo[:, :, ts(i, M)])
                        nc.sync.dma_start(oh_t[:, LH:, :], oh_hi[:, :, ts(i, M)])

                # xs[l] = x[l-1] + x[l+1] for interior l (one stacked DVE op);
                # boundary layers use the single neighbor directly.
                xs_t = xp.tile([C, L - 2, M], BF16, tag="xs")
                nc.vector.tensor_tensor(
                    xs_t[:], xt_t[:, 0 : L - 2, :], xt_t[:, 2:L, :],
                    mybir.AluOpType.add,
                )

                # z1[l] (pre-relu) accumulates directly in a PSUM bank:
                #   W1'^T x[l] + (Wtx@W1)^T (x[l-1]+x[l+1]) + (T@W1)^T ohsum[l]
                y_ps = pp.tile([C, M], F32, tag="y", bufs=2)
                out_t = outp.tile([C, M], BF16, tag="out")
                r_ts = []
                # relu engine per layer (GPSIMD can't read PSUM: ACT/DVE only)
                relu_eng = ("act", "dve", "act", "dve", "act", "dve")
                for l in range(L):
                    z_ps = pp.tile([C, M], F32, tag="z1", bufs=6)
                    nc.tensor.matmul(
                        z_ps[:], w1d_sb[:], xt_t[:, l, :], start=True, stop=False
                    )
                    nbr = (
                        xt_t[:, 1, :] if l == 0
                        else xt_t[:, L - 2, :] if l == L - 1
                        else xs_t[:, l - 1, :]
                    )
                    nc.tensor.matmul(z_ps[:], w1x_sb[:], nbr, start=False, stop=False)
                    if DOUBLE_ROW:
                        nc.tensor.matmul(
                            z_ps[:], tw1_sb[:], oh_t[:, :, l, :],
                            start=False, stop=True,
                            perf_mode=mybir.MatmulPerfMode.DoubleRow,
                        )
                    else:
                        nc.tensor.matmul(
                            z_ps[:], tw1_sb[:], oh_t[:, l, :],
                            start=False, stop=True,
                        )
                    r_t = rp.tile([C, M], BF16, tag=f"r{l}")
                    eng = relu_eng[l]
                    if eng == "act":
                        nc.scalar.activation(r_t[:], z_ps[:], RELU, bias=b1_sb[:])
                    elif eng == "dve":
                        nc.vector.tensor_scalar(
                            r_t[:], z_ps[:], b1_sb[:], 0.0,
                            mybir.AluOpType.add, mybir.AluOpType.max,
                        )
                    else:
                        nc.gpsimd.tensor_scalar(
                            r_t[:], z_ps[:], b1_sb[:], 0.0,
                            mybir.AluOpType.add, mybir.AluOpType.max,
                        )
                    r_ts.append(r_t)

                if i_rep == repeat * nt - 1:
                    # last tile: per-layer W2 accumulation (a little more PE,
                    # but drops the serial Pool add-tree from the drain path)
                    for l in range(L):
                        nc.tensor.matmul(
                            y_ps[:], w2_sb[:], r_ts[l][:],
                            start=(l == 0), stop=(l == L - 1),
                        )
                else:
                    # 5-add bf16 tree on Pool (SBUF-only engine; W2
                    # distributes over the layer sum), then a single W2
                    # matmul into the y PSUM bank
                    s0 = rp.tile([C, M], BF16, tag="s0")
                    nc.gpsimd.tensor_tensor(
                        s0[:], r_ts[0][:], r_ts[1][:], mybir.AluOpType.add
                    )
                    s1 = rp.tile([C, M], BF16, tag="s1")
                    nc.gpsimd.tensor_tensor(
                        s1[:], r_ts[2][:], r_ts[3][:], mybir.AluOpType.add
                    )
                    s2 = rp.tile([C, M], BF16, tag="s2")
                    nc.gpsimd.tensor_tensor(
                        s2[:], r_ts[4][:], r_ts[5][:], mybir.AluOpType.add
                    )
                    t0 = rp.tile([C, M], BF16, tag="t0")
                    nc.gpsimd.tensor_tensor(
                        t0[:], s0[:], s1[:], mybir.AluOpType.add
                    )
                    rsum = rp.tile([C, M], BF16, tag="rsum")
                    nc.gpsimd.tensor_tensor(
                        rsum[:], t0[:], s2[:], mybir.AluOpType.add
                    )
                    nc.tensor.matmul(
                        y_ps[:], w2_sb[:], rsum[:], start=True, stop=True
                    )
                nc.scalar.activation(out_t[:], y_ps[:], IDENT, bias=b2_sb[:])
                nc.gpsimd.dma_start(out[:, ts(i, M)], out_t[:])

    nc.compile()
    return nc


def prep_host(x, atomic_type, emb, Wt, bt, eps, W1, b1, W2, b2, nc_n=NC_N,
              ncores=NCORES):
    """Host-side prep: fold eps into weights, build per-core input maps."""
    x = np.asarray(x, dtype=np.float32)
    idx = np.asarray(atomic_type).astype(np.int64)
    emb = np.asarray(emb, dtype=np.float32)
    Wt = np.asarray(Wt, dtype=np.float32)
    bt = np.asarray(bt, dtype=np.float32)
    W1 = np.asarray(W1, dtype=np.float32)
    b1 = np.asarray(b1, dtype=np.float32)
    W2 = np.asarray(W2, dtype=np.float32)
    b2 = np.asarray(b2, dtype=np.float32)
    scale = 1.0 + np.float32(np.asarray(eps).reshape(-1)[0])

    # W1 folded through the propagate step (eps-scales cancel in the products):
    #   z1[l] = x[l] @ (scale*W1) + x[l+/-1] @ (Wt[:C] @ W1) + ohsum[l] @ (T @ W1)
    # with T = emb @ Wt[C:] + bt.
    T = (emb @ Wt[C:]) + bt  # [EMB, C]
    w1d = np.ascontiguousarray((W1 * scale).astype(ml_dtypes.bfloat16))
    w1x = np.ascontiguousarray(
        (Wt[:C].astype(np.float64) @ W1.astype(np.float64)).astype(
            ml_dtypes.bfloat16
        )
    )
    tw1 = (_OH_SCALE * (T.astype(np.float64) @ W1.astype(np.float64))).astype(
        ml_dtypes.float8_e4m3
    )
    if DOUBLE_ROW:
        tw1 = np.ascontiguousarray(tw1.reshape(EMB_H, 2, C))
    w2s = np.ascontiguousarray(W2.astype(ml_dtypes.bfloat16))
    b1c = np.ascontiguousarray(b1.reshape(C, 1))
    b2s = np.ascontiguousarray((np.float32(L) * b2).reshape(C, 1))

    arange_emb = np.arange(EMB, dtype=idx.dtype)
    in_maps = []
    for k in range(ncores):
        n0 = k * nc_n
        xs = x[:, n0 : n0 + nc_n, :]  # [L, nc_n, C]
        xtk = np.ascontiguousarray(xs.transpose(2, 0, 1)).astype(
            ml_dtypes.bfloat16
        )  # [C, L, nc_n]
        ii = idx[:, n0 : n0 + nc_n]  # [L, nc_n]
        ohb = (ii[:, None, :] == arange_emb[None, :, None]).view(np.uint8)
        ohs = np.zeros((L, EMB, nc_n), dtype=np.uint8)
        ohs[:-1] += ohb[1:]
        ohs[1:] += ohb[:-1]
        ohk = _FP8_LUT[ohs.transpose(1, 0, 2)]  # [EMB, L, nc_n] uint8 bits
        ohk = np.ascontiguousarray(ohk).view(ml_dtypes.float8_e4m3)
        if DOUBLE_ROW:
            ohk = ohk.reshape(EMB_H, 2, L, nc_n)
        in_maps.append(
            {
                "xt": xtk,
                "oh": ohk,
                "w1d": w1d,
                "w1x": w1x,
                "tw1": tw1,
                "w2": w2s,
                "b1": b1c,
                "b2s": b2s,
            }
        )
    return in_maps


_COMPILED = {}


def get_compiled(nc_n=NC_N, num_devices=NCORES):
    key = (nc_n, num_devices)
    if key not in _COMPILED:
        _COMPILED[key] = build_bass(nc_n, num_devices)
    return _COMPILED[key]


def run_on_hw(in_maps, nc=None, trace=False, **kwargs):
    if nc is None:
        nc = get_compiled()
    return bass_utils.run_bass_kernel_spmd(
        nc, in_maps, core_ids=list(range(len(in_maps))), trace=trace, **kwargs
    )


def kernel(**inputs) -> np.ndarray:
    in_maps = prep_host(
        inputs["x"],
        inputs["atomic_type"],
        inputs["emb"],
        inputs["Wt"],
        inputs["bt"],
        inputs["eps"],
        inputs["W1"],
        inputs["b1"],
        inputs["W2"],
        inputs["b2"],
    )
    res = run_on_hw(in_maps)
    out = np.empty((N_FULL, C), dtype=np.float32)
    for k in range(NCORES):
        out[k * NC_N : (k + 1) * NC_N, :] = (
            res.results[k]["out"].astype(np.float32).T
        )
    return out


if __name__ == "__main__":
    import reference  # only when run manually inside /root/problem

    inputs = {k: np.asarray(v) for k, v in reference.setup_inputs().items()}
    got = kernel(**inputs)
    want = np.asarray(reference.reference(**inputs))
    err = np.abs(got - want).max() / np.abs(want).max()
    print("rel err:", err)
